# revision 20
# baseline (speedup 1.0000x reference)
"""TRN2 Bass/Tile kernel for AlignmentAttention (gated dot + additive/Bahdanau attention).

Problem (hardcoded shapes): B=4, H=8, T=S=256, DK=64, fp32.
  dot branch : softmax(mask(Q@K.T/8)) @ V, softmax probs are also an output (dot_score).
  add branch : softmax(mask(sum_e v_w[e]*tanh(qp[t,e]+kp[s,e]))) @ V,
               qp = Q@Wq.T, kp = K@Wk.T.
  gate       : sigmoid(Q@Wg.T + bg);  out = gate*dot + (1-gate)*add.

Sharding: batch*heads (32) split across 8 cores -> 4 heads per core, one batch b
per core (core c -> b=c//2, heads 4*(c%2)..+4), so the [T,S] mask is shared per core.

Per-head on-chip plan (all layouts note [partition, free]):
  - Q,K loaded [t|s,64] then PE-transposed to QT65 [65(d+ones),256], KT [64,256].
  - qpT2 [128,256] = [Wq.T|Wq.T].T-matmul -> (qp.T stacked twice on partitions).
  - kp_pairs [128,128]: col j = [kp[2j,:];kp[2j+1,:]] via two zero-padded matmuls.
  - z-chunks: DVE tensor_scalar_add(qpT2 + kp_pairs col) -> [128, G*256];
    one big ACT tanh per chunk; then per pair an N=2 matmul with
    rhs=[v_w;0 | 0;v_w] gives add_sc [t,2s] slabs into PSUM [128,2,256].
  - masked softmax in [t,s] with (mask-1)*BIG additive tiles; exp on ACT with
    per-partition -max bias and free row-sum via accum_out.
  - probs PE-transposed [s,t], @V with PSUM accumulation; unnormalized for the
    add branch (scaled by 1/rowsum at combine time).
"""

import numpy as np
from contextlib import ExitStack

import concourse.bass as bass
import concourse.bacc as bacc
import concourse.tile as tile
from concourse import masks, mybir
from concourse.bass_utils import run_bass_kernel_spmd

B, H, T, S, DK = 4, 8, 256, 256, 64
N_CORES = 8
HPC = (B * H) // N_CORES  # heads per core
P = 128
NEG = 1.0e8  # reference masks with -1e8 before softmax
F32 = mybir.dt.float32

# knobs
TH_DT = mybir.dt.float32  # dtype of tanh output (lhsT of the sum_e matmuls)
G = 16  # s-pairs per z/tanh chunk -> ACT insts of [128, G*256]
Z_GPS_FRAC = 0.45  # fraction of z-build tensor_scalar ops routed to GPSIMD

AF = mybir.ActivationFunctionType
AX = mybir.AxisListType
ALU = mybir.AluOpType


def ts(i, size):
    return slice(i * size, (i + 1) * size)


def build_nc(th_dt=TH_DT, g_chunk=G, n_heads=HPC, stage=99):
    nc = bacc.Bacc(None, target_bir_lowering=False)

    q_d = nc.dram_tensor("q", [n_heads, T, DK], F32, kind="ExternalInput")
    k_d = nc.dram_tensor("k", [n_heads, S, DK], F32, kind="ExternalInput")
    v_d = nc.dram_tensor("v", [n_heads, S, DK], F32, kind="ExternalInput")
    negd_d = nc.dram_tensor("negd", [P, 2, S], F32, kind="ExternalInput")
    nega_d = nc.dram_tensor("nega", [P, 2, S], F32, kind="ExternalInput")
    wqt2_d = nc.dram_tensor("wqt2", [DK, 2 * DK], F32, kind="ExternalInput")
    wkt2a_d = nc.dram_tensor("wkt2a", [DK, 2 * DK], F32, kind="ExternalInput")
    wkt2b_d = nc.dram_tensor("wkt2b", [DK, 2 * DK], F32, kind="ExternalInput")
    consts_d = nc.dram_tensor("consts", [P, 8], F32, kind="ExternalInput")
    ident_d = nc.dram_tensor("ident", [P, P], F32, kind="ExternalInput")
    out_d = nc.dram_tensor("out", [n_heads, T, DK], F32, kind="ExternalOutput")
    ds_d = nc.dram_tensor("ds", [n_heads, T, S], F32, kind="ExternalOutput")

    with tile.TileContext(nc) as tc, ExitStack() as ctx:
        singles = ctx.enter_context(tc.tile_pool(name="singles", bufs=1))
        io = ctx.enter_context(tc.tile_pool(name="io", bufs=5))
        tr = ctx.enter_context(tc.tile_pool(name="tr", bufs=5))
        prep = ctx.enter_context(tc.tile_pool(name="prep", bufs=5))
        zpool = ctx.enter_context(tc.tile_pool(name="zpool", bufs=3))
        thpool = ctx.enter_context(tc.tile_pool(name="thpool", bufs=3))
        soft = ctx.enter_context(tc.tile_pool(name="soft", bufs=3))
        ppool = ctx.enter_context(tc.tile_pool(name="ppool", bufs=6))
        ptpool = ctx.enter_context(tc.tile_pool(name="ptpool", bufs=4))
        outp = ctx.enter_context(tc.tile_pool(name="outp", bufs=3))
        small = ctx.enter_context(tc.tile_pool(name="small", bufs=10))

        ps_addsc = ctx.enter_context(tc.tile_pool(name="ps_addsc", bufs=2, space="PSUM"))
        ps_sh = ctx.enter_context(tc.tile_pool(name="ps_sh", bufs=2, space="PSUM"))
        ps_tr = ctx.enter_context(tc.tile_pool(name="ps_tr", bufs=2, space="PSUM"))
        ps_pv = ctx.enter_context(tc.tile_pool(name="ps_pv", bufs=2, space="PSUM"))

        # ---- one-time setup: everything host-precomputed, just DMA in ----
        # (head-0 q/k/v loads are emitted first inside the prep loop; keep the
        #  identity early since the first transposes need it)
        ident = singles.tile([P, P], F32, tag="ident")
        nc.scalar.dma_start(out=ident[:], in_=ident_d[:, :])
        wqT2 = singles.tile([DK, 2 * DK], F32, tag="wqT2")
        nc.scalar.dma_start(out=wqT2[:], in_=wqt2_d[:, :])
        wkT2a = singles.tile([DK, 2 * DK], F32, tag="wkT2a")
        nc.scalar.dma_start(out=wkT2a[:], in_=wkt2a_d[:, :])
        wkT2b = singles.tile([DK, 2 * DK], F32, tag="wkT2b")
        nc.scalar.dma_start(out=wkT2b[:], in_=wkt2b_d[:, :])
        consts = singles.tile([P, 8], F32, tag="consts")
        nc.scalar.dma_start(out=consts[:], in_=consts_d[:, :])
        vw2 = consts[:, 0:2]
        if th_dt != F32:
            vw2c_t = singles.tile([P, 2], th_dt, tag="vw2c")
            nc.vector.tensor_copy(out=vw2c_t[:], in_=vw2)
            vw2c = vw2c_t[:]
        else:
            vw2c = vw2
        wg_col = consts[0:DK, 2:3]   # gate weight as matmul rhs [64, 1]
        bg_col = consts[:, 3:4]      # bg replicated [128, 1]
        negd = singles.tile([P, 2, S], F32, tag="negd")
        nc.scalar.dma_start(out=negd[:], in_=negd_d[:, :])
        nega = singles.tile([P, 2, S], F32, tag="nega")
        nc.scalar.dma_start(out=nega[:], in_=nega_d[:, :])

        n_pairs = S // 2

        prep_st = []
        for h in range(n_heads):
            # ---- loads ----
            q_sb = io.tile([P, 2, DK], F32, tag="q")
            k_sb = io.tile([P, 2, DK], F32, tag="k")
            v_sb = io.tile([P, 2, DK], F32, tag="v")
            nc.sync.dma_start(out=q_sb[:], in_=q_d[h].rearrange("(i p) d -> p i d", p=P))
            nc.sync.dma_start(out=k_sb[:], in_=k_d[h].rearrange("(i p) d -> p i d", p=P))
            nc.sync.dma_start(out=v_sb[:], in_=v_d[h].rearrange("(i p) d -> p i d", p=P))

            # ---- transposes: QT65 [65, 256] (row 64 = ones), KT [64, 256] ----
            qt65 = tr.tile([DK + 1, T], F32, tag="qt")
            kt = tr.tile([DK, S], F32, tag="kt")
            nc.vector.memset(qt65[DK : DK + 1, :], 1.0)
            for i in range(2):
                tp_q = ps_tr.tile([P, P], F32, tag="tr")
                nc.tensor.transpose(tp_q[0:DK, :], q_sb[:, i, :], ident[:])
                nc.vector.tensor_copy(out=qt65[0:DK, ts(i, P)], in_=tp_q[0:DK, :])
                tp_k = ps_tr.tile([P, P], F32, tag="tr")
                nc.tensor.transpose(tp_k[0:DK, :], k_sb[:, i, :], ident[:])
                nc.vector.tensor_copy(out=kt[:, ts(i, P)], in_=tp_k[0:DK, :])

            if stage < 2:
                continue
            # ---- qpT2 [128, 256]: rows 0:64 = qp.T, rows 64:128 = qp.T ----
            qp_ps = ps_sh.tile([P, 512], F32, tag="sh")
            nc.tensor.matmul(qp_ps[:, 0:T], lhsT=wqT2[:], rhs=qt65[0:DK, :],
                             start=True, stop=True)
            qpT2 = prep.tile([P, T], F32, tag="qp")
            nc.vector.tensor_copy(out=qpT2[:], in_=qp_ps[:, 0:T])

            # ---- kp_pairs [128, 128]: col j = [kp[2j,:]; kp[2j+1,:]] ----
            kt_pairs = kt[:].rearrange("d (j two) -> d j two", two=2)
            kp_ps = ps_sh.tile([P, 512], F32, tag="sh")
            nc.tensor.matmul(kp_ps[:, 0:n_pairs], lhsT=wkT2a[:],
                             rhs=kt_pairs[:, :, 0], start=True, stop=False)
            nc.tensor.matmul(kp_ps[:, 0:n_pairs], lhsT=wkT2b[:],
                             rhs=kt_pairs[:, :, 1], start=False, stop=True)
            kp_pairs = prep.tile([P, n_pairs], F32, tag="kp")
            nc.vector.tensor_copy(out=kp_pairs[:], in_=kp_ps[:, 0:n_pairs])
            prep_st.append((q_sb, k_sb, v_sb, qt65, kt, qpT2, kp_pairs))

        for h in range(n_heads):
            if stage < 3:
                continue
            q_sb, k_sb, v_sb, qt65, kt, qpT2, kp_pairs = prep_st[h]
            # ---- dot scores + gate ----
            sc_ps = ps_sh.tile([P, 512], F32, tag="sh")
            for i in range(2):
                nc.tensor.matmul(sc_ps[:, slice(i * S, (i + 1) * S)],
                                 lhsT=qt65[0:DK, ts(i, P)], rhs=kt[:],
                                 start=True, stop=True)

            if stage < 6:
                continue
            # ---- additive branch: z chunks -> tanh -> sum_e matmuls ----
            addsc_ps = ps_addsc.tile([P, 2, S], F32, tag="addsc")
            n_gps = int(round(g_chunk * Z_GPS_FRAC))
            for c in range(n_pairs // g_chunk):
                zbuf = zpool.tile([P, g_chunk, T], F32, tag="z")
                for g in range(g_chunk):
                    j = c * g_chunk + g
                    eng = nc.gpsimd if (g < n_gps and not (h == 0 and c == 0)) else nc.vector
                    eng.tensor_scalar_add(zbuf[:, g, :], qpT2[:],
                                          kp_pairs[:, j : j + 1])
                thbuf = thpool.tile([P, g_chunk, T], th_dt, tag="th")
                if h == 0 and c == 0:
                    # split the very first tanh so ACT starts sooner
                    q4 = g_chunk // 4
                    for m in range(4):
                        nc.scalar.activation(out=thbuf[:, ts(m, q4), :],
                                             in_=zbuf[:, ts(m, q4), :], func=AF.Tanh)
                else:
                    nc.scalar.activation(out=thbuf[:], in_=zbuf[:], func=AF.Tanh)
                for g in range(g_chunk):
                    j = c * g_chunk + g
                    for i in range(2):
                        nc.tensor.matmul(addsc_ps[:, i, 2 * j : 2 * j + 2],
                                         lhsT=thbuf[:, g, ts(i, P)], rhs=vw2c[:],
                                         start=True, stop=True)

            g_ps = ps_pv.tile([P, DK], F32, tag="pv")
            for i in range(2):
                nc.tensor.matmul(g_ps[:, i : i + 1], lhsT=qt65[0:DK, ts(i, P)],
                                 rhs=wg_col, start=True, stop=True)
            g_sb = small.tile([P, 2], F32, tag="g_sb")
            nc.vector.tensor_copy(out=g_sb[:], in_=g_ps[:, 0:2])
            gneg = small.tile([P, 2], F32, tag="gneg")
            nc.vector.tensor_scalar(out=gneg[:], in0=g_sb[:], scalar1=bg_col,
                                    scalar2=-1.0, op0=ALU.add, op1=ALU.mult)
            eg = small.tile([P, 2], F32, tag="eg")
            nc.scalar.activation(out=eg[:], in_=gneg[:], func=AF.Exp)
            egp1 = small.tile([P, 2], F32, tag="egp1")
            nc.vector.tensor_scalar_add(egp1[:], eg[:], 1.0)
            gate = small.tile([P, 2], F32, tag="gate")
            nc.vector.reciprocal(gate[:], egp1[:])

            if stage < 4:
                continue
            # ---- dot masked softmax (scale 1/8 folded into exp) ----
            pd_t, rd_t = [], []
            for i in range(2):
                mscd = soft.tile([P, S], F32, tag="msc")
                nc.vector.tensor_add(mscd[:], sc_ps[:, slice(i * S, (i + 1) * S)],
                                     negd[:, i, :])
                mx = small.tile([P, 1], F32, tag="mx")
                nc.vector.reduce_max(mx[:], mscd[:], axis=AX.X)
                msh = soft.tile([P, S], F32, tag="msh")
                nc.vector.tensor_scalar_sub(msh[:], mscd[:], mx[:])
                pd = ppool.tile([P, S], F32, tag="pd")
                rs = small.tile([P, 1], F32, tag="rsd")
                nc.scalar.activation(out=pd[:], in_=msh[:], func=AF.Exp,
                                     scale=0.125, accum_out=rs[:])
                rd = small.tile([P, 1], F32, tag="rd")
                nc.vector.reciprocal(rd[:], rs[:])
                dsout = outp.tile([P, S], F32, tag="ds")
                nc.vector.tensor_scalar_mul(dsout[:], pd[:], rd[:])
                nc.sync.dma_start(out=ds_d[h, ts(i, P), :], in_=dsout[:])
                pd_t.append(pd)
                rd_t.append(rd)

            if stage < 5:
                continue
            # ---- dot probs transpose (PV deferred to the combine loop) ----
            pdT = []
            for j in range(2):
                pt = ptpool.tile([P, T], F32, tag="pT")
                for i in range(2):
                    tp = ps_tr.tile([P, P], F32, tag="tr")
                    nc.tensor.transpose(tp[:], pd_t[i][:, ts(j, P)], ident[:])
                    nc.vector.tensor_copy(out=pt[:, ts(i, P)], in_=tp[:])
                pdT.append(pt)

            if stage < 7:
                continue
            # ---- additive masked softmax ----
            pa_t, ra_t = [], []
            for i in range(2):
                msca = soft.tile([P, S], F32, tag="msc")
                nc.vector.tensor_add(msca[:], addsc_ps[:, i, :], nega[:, i, :])
                mxa = small.tile([P, 1], F32, tag="mxa")
                nc.vector.reduce_max(mxa[:], msca[:], axis=AX.X)
                msha = soft.tile([P, S], F32, tag="msh")
                nc.vector.tensor_scalar_sub(msha[:], msca[:], mxa[:])
                pa = ppool.tile([P, S], F32, tag="pa")
                rsa = small.tile([P, 1], F32, tag="rsa")
                nc.scalar.activation(out=pa[:], in_=msha[:], func=AF.Exp,
                                     accum_out=rsa[:])
                ra = small.tile([P, 1], F32, tag="ra")
                nc.vector.reciprocal(ra[:], rsa[:])
                pa_t.append(pa)
                ra_t.append(ra)

            if stage < 8:
                continue
            # ---- additive probs transpose + @V ----
            paT = []
            for j in range(2):
                pt = ptpool.tile([P, T], F32, tag="pT")
                for i in range(2):
                    tp = ps_tr.tile([P, P], F32, tag="tr")
                    nc.tensor.transpose(tp[:], pa_t[i][:, ts(j, P)], ident[:])
                    nc.vector.tensor_copy(out=pt[:, ts(i, P)], in_=tp[:])
                paT.append(pt)

            for i in range(2):
                pvd = ps_pv.tile([P, DK], F32, tag="pv")
                for j in range(2):
                    nc.tensor.matmul(pvd[:], lhsT=pdT[j][:, ts(i, P)], rhs=v_sb[:, j, :],
                                     start=(j == 0), stop=(j == 1))
                pva = ps_pv.tile([P, DK], F32, tag="pv")
                for j in range(2):
                    nc.tensor.matmul(pva[:], lhsT=paT[j][:, ts(i, P)], rhs=v_sb[:, j, :],
                                     start=(j == 0), stop=(j == 1))

                # ---- combine: out = gate*dot + (1-gate)*add, with
                # dot = pvd*rd, add = pva*ra, (1-gate) = eg*gate ----
                cd = small.tile([P, 1], F32, tag="cd")
                nc.vector.tensor_scalar_mul(cd[:], rd_t[i][:], gate[:, i : i + 1])
                ca0 = small.tile([P, 1], F32, tag="ca0")
                nc.vector.tensor_scalar_mul(ca0[:], ra_t[i][:], gate[:, i : i + 1])
                ca = small.tile([P, 1], F32, tag="ca")
                nc.vector.tensor_scalar_mul(ca[:], ca0[:], eg[:, i : i + 1])
                pvd_sb = outp.tile([P, DK], F32, tag="pvd_sb")
                nc.vector.tensor_copy(out=pvd_sb[:], in_=pvd[:])
                pva_sb = outp.tile([P, DK], F32, tag="pva_sb")
                nc.vector.tensor_copy(out=pva_sb[:], in_=pva[:])
                o1 = outp.tile([P, DK], F32, tag="o1")
                nc.vector.tensor_scalar_mul(o1[:], pvd_sb[:], cd[:])
                o2 = outp.tile([P, DK], F32, tag="o2")
                nc.vector.tensor_scalar_mul(o2[:], pva_sb[:], ca[:])
                osb = outp.tile([P, DK], F32, tag="osb")
                nc.vector.tensor_add(osb[:], o1[:], o2[:])
                nc.sync.dma_start(out=out_d[h, ts(i, P), :], in_=osb[:])

    nc.compile()
    return nc


_NC_CACHE = {}


def _get_nc():
    key = (TH_DT, G)
    if key not in _NC_CACHE:
        _NC_CACHE[key] = build_nc(TH_DT, G)
    return _NC_CACHE[key]


def make_in_maps(query, key, value, mask, Wq, Wk, v_w, Wg, bg):
    query = np.ascontiguousarray(np.asarray(query, dtype=np.float32))
    key = np.ascontiguousarray(np.asarray(key, dtype=np.float32))
    value = np.ascontiguousarray(np.asarray(value, dtype=np.float32))
    mask = np.asarray(mask)
    Wq = np.asarray(Wq, dtype=np.float32)
    Wk = np.asarray(Wk, dtype=np.float32)
    vw = np.asarray(v_w, dtype=np.float32).reshape(DK)
    wg = np.asarray(Wg, dtype=np.float32).reshape(DK)
    bgv = np.asarray(bg, dtype=np.float32).reshape(1)

    wqt2 = np.ascontiguousarray(np.concatenate([Wq.T, Wq.T], axis=1))
    z64 = np.zeros((DK, DK), np.float32)
    wkt2a = np.ascontiguousarray(np.concatenate([Wk.T, z64], axis=1))
    wkt2b = np.ascontiguousarray(np.concatenate([z64, Wk.T], axis=1))
    consts = np.zeros((P, 8), np.float32)
    consts[0:DK, 0] = vw
    consts[DK:P, 1] = vw
    consts[0:DK, 2] = wg
    consts[:, 3] = bgv[0]
    ident = np.eye(P, dtype=np.float32)

    in_maps = []
    for c in range(N_CORES):
        b = c // 2
        h0 = (c % 2) * HPC
        mf = mask[b, 0].astype(np.float32)  # [T, S]
        negd = ((mf - 1.0) * (8.0 * NEG)).reshape(2, P, S).transpose(1, 0, 2)
        nega = ((mf - 1.0) * NEG).reshape(2, P, S).transpose(1, 0, 2)
        in_maps.append({
            "q": np.ascontiguousarray(query[b, h0 : h0 + HPC]),
            "k": np.ascontiguousarray(key[b, h0 : h0 + HPC]),
            "v": np.ascontiguousarray(value[b, h0 : h0 + HPC]),
            "negd": np.ascontiguousarray(negd),
            "nega": np.ascontiguousarray(nega),
            "wqt2": wqt2,
            "wkt2a": wkt2a,
            "wkt2b": wkt2b,
            "consts": consts,
            "ident": ident,
        })
    return in_maps


def assemble(results):
    out = np.empty((B, H, T, DK), np.float32)
    ds = np.empty((B, H, T, S), np.float32)
    for c in range(N_CORES):
        b = c // 2
        h0 = (c % 2) * HPC
        out[b, h0 : h0 + HPC] = results[c]["out"]
        ds[b, h0 : h0 + HPC] = results[c]["ds"]
    return out, ds


def run(inputs, **spmd_kwargs):
    nc = _get_nc()
    in_maps = make_in_maps(**inputs)
    res = run_bass_kernel_spmd(nc, in_maps, core_ids=list(range(N_CORES)), **spmd_kwargs)
    return res


def kernel(**inputs):
    res = run(inputs)
    return assemble(res.results)
